# revision 9
# baseline (speedup 1.0000x reference)
"""Bass/Trainium2 kernel for nn_Encoder (embedding -> BiLSTM -> cross attention -> enhancement).

Sharding: data-parallel over batch, 16 items per core on 8 NeuronCores.
Per core the A and B sequences are stacked into 32 rows; fwd+bwd LSTM
directions are fused into a single 64-row elementwise stream per timestep
(fwd rows 0:32, bwd rows 32:64) with the two direction matmul groups placed
on different PE column groups.

LSTM matmuls run in float32r (full-rate fp32 PE mode); elementwise math and
the (small-N) attention matmuls in fp32.
"""

import numpy as np

V, E, H = 32000, 300, 512
BSZ, T = 128, 128
NCORES = 8
PB = BSZ // NCORES          # 16 batch items per core
RW = 2 * PB                 # 32 stacked rows (A items then B items)
RT = 2 * RW                 # 64 rows in fused fwd+bwd elementwise space
G4 = 4 * H                  # 2048 gate width
H2 = 2 * H                  # 1024 bilstm output width
KCH = [(0, 128), (128, 128), (256, 44)]   # chunks of E=300

_CACHE = {}


def _build():
    import concourse.mybir as mybir
    import concourse.tile as tile
    from concourse import bacc
    from concourse.masks import make_identity

    F32 = mybir.dt.float32
    F32R = mybir.dt.float32r
    AF = mybir.ActivationFunctionType
    ALU = mybir.AluOpType
    AX = mybir.AxisListType

    nc = bacc.Bacc("TRN2", target_bir_lowering=False, debug=False,
                   num_devices=NCORES)

    xT_d = nc.dram_tensor("xT", [E, RW * T], F32R, kind="ExternalInput")
    wih_d = {d: nc.dram_tensor(f"wihT_{d}", [E, G4], F32R, kind="ExternalInput")
             for d in "fb"}
    whh_d = {d: nc.dram_tensor(f"whhT_{d}", [H, G4], F32R, kind="ExternalInput")
             for d in "fb"}
    bias_d = {d: nc.dram_tensor(f"bias_{d}", [128, G4], F32, kind="ExternalInput")
              for d in "fb"}
    outA_d = nc.dram_tensor("outA", [PB, T, 4 * H2], F32, kind="ExternalOutput")
    outB_d = nc.dram_tensor("outB", [PB, T, 4 * H2], F32, kind="ExternalOutput")

    with tile.TileContext(nc) as tc:
        with tc.tile_pool(name="dram", bufs=1, space="DRAM") as dpool, \
             tc.tile_pool(name="const", bufs=1) as const:
            xw = {d: dpool.tile([RW, T, G4], F32, name=f"xw_{d}") for d in "fb"}
            tm = dpool.tile([RW, T, H2], F32)
            ident = const.tile([128, 128], F32)
            make_identity(nc, ident[:])

            # ---------------- Phase 1: input projections ----------------
            with tc.tile_pool(name="p1w", bufs=1) as p1w, \
                 tc.tile_pool(name="p1ps", bufs=2, space="PSUM") as p1ps, \
                 tc.tile_pool(name="p1e", bufs=3) as p1e:
                xT_sb = []
                for ki, (ko, ks) in enumerate(KCH):
                    t_ = p1w.tile([ks, RW * T], F32R, tag=f"xT{ki}")
                    nc.sync.dma_start(t_[:], xT_d.ap()[ko:ko + ks, :])
                    xT_sb.append(t_)
                for d in "fb":
                    wih_sb = []
                    for ki, (ko, ks) in enumerate(KCH):
                        t_ = p1w.tile([ks, G4], F32R, tag=f"wih{d}{ki}")
                        nc.sync.dma_start(t_[:], wih_d[d].ap()[ko:ko + ks, :])
                        wih_sb.append(t_)
                    bias_sb = p1w.tile([128, G4], F32, tag=f"bias{d}")
                    nc.sync.dma_start(bias_sb[:], bias_d[d].ap())
                    for rc in range(RW):
                        ps = p1ps.tile([128, G4], F32, tag="pj")
                        for nj in range(4):
                            for ki in range(3):
                                nc.tensor.matmul(
                                    ps[:, nj * 512:(nj + 1) * 512],
                                    xT_sb[ki][:, rc * T:(rc + 1) * T],
                                    wih_sb[ki][:, nj * 512:(nj + 1) * 512],
                                    start=(ki == 0), stop=(ki == 2))
                        ev = p1e.tile([128, G4], F32, tag="ev")
                        nc.vector.tensor_add(ev[:], ps[:], bias_sb[:])
                        nc.sync.dma_start(xw[d][rc, :, :], ev[:])

            # ---------------- Phase 2: bidirectional LSTM scan ----------------
            with tc.tile_pool(name="wst", bufs=1) as wst, \
                 tc.tile_pool(name="sst", bufs=1) as sst, \
                 tc.tile_pool(name="xwp", bufs=2) as xwp, \
                 tc.tile_pool(name="gp", bufs=2) as gp, \
                 tc.tile_pool(name="gps", bufs=1, space="PSUM") as gps_pool, \
                 tc.tile_pool(name="tps", bufs=2, space="PSUM") as tps_pool:
                whh_sb = {}
                for d in "fb":
                    whh_sb[d] = []
                    for kc in range(4):
                        w = wst.tile([128, G4], F32R, tag=f"whh{d}{kc}")
                        nc.sync.dma_start(w[:], whh_d[d].ap()[kc * 128:(kc + 1) * 128, :])
                        whh_sb[d].append(w)
                # hT_d: transposed h state per direction; chunk c in cols [32c:32c+32]
                hT = {d: sst.tile([128, 4 * RW], F32R, name=f"hT_{d}") for d in "fb"}
                c_st = {d: sst.tile([RW, H], F32, name=f"c_st_{d}") for d in "fb"}

                for t in range(T):
                    for di, d in enumerate("fb"):
                        tx = t if d == "f" else T - 1 - t
                        xwt = xwp.tile([RW, G4], F32, tag=f"xwt{d}")
                        nc.sync.dma_start(xwt[:], xw[d][:, tx, :])
                        # gates layout (host permuted): [i | f | o | g]
                        g_ifo = gp.tile([RW, 3 * H], F32, tag=f"g_ifo{d}")
                        g_g = gp.tile([RW, H], F32, tag=f"g_g{d}")
                        if t == 0:
                            # h == 0: gates are just xw + bias (bias folded in xw)
                            nc.vector.tensor_copy(g_ifo[:], xwt[:, 0:3 * H])
                            nc.vector.tensor_copy(g_g[:], xwt[:, 3 * H:G4])
                        else:
                            gps = gps_pool.tile([RW, G4], F32, tag=f"g{d}")
                            for nj in range(4):
                                for kc in range(4):
                                    nc.tensor.matmul(
                                        gps[:, nj * 512:(nj + 1) * 512],
                                        hT[d][:, 32 * kc:32 * kc + RW],
                                        whh_sb[d][kc][:, nj * 512:(nj + 1) * 512],
                                        start=(kc == 0), stop=(kc == 3))
                            nc.vector.tensor_add(g_ifo[:], gps[:, 0:3 * H],
                                                 xwt[:, 0:3 * H])
                            nc.vector.tensor_add(g_g[:], gps[:, 3 * H:G4],
                                                 xwt[:, 3 * H:G4])
                        sg = g_ifo
                        nc.scalar.activation(sg[:], g_ifo[:], AF.Sigmoid)
                        tg = g_g
                        nc.scalar.activation(tg[:], g_g[:], AF.Tanh)
                        p_ = gp.tile([RW, H], F32, tag=f"p_{d}")
                        nc.gpsimd.tensor_mul(p_[:], sg[:, 0:H], tg[:])
                        if t == 0:
                            nc.vector.tensor_copy(c_st[d][:], p_[:])
                        else:
                            q_ = gp.tile([RW, H], F32, tag=f"q_{d}")
                            nc.gpsimd.tensor_mul(q_[:], sg[:, H:2 * H], c_st[d][:])
                            nc.vector.tensor_add(c_st[d][:], p_[:], q_[:])
                        th = gp.tile([RW, H], F32, tag=f"th{d}")
                        nc.scalar.activation(th[:], c_st[d][:], AF.Tanh)
                        h_ = gp.tile([RW, H], F32, tag=f"h_{d}")
                        nc.vector.tensor_mul(h_[:], sg[:, 2 * H:3 * H], th[:])
                        tp = gps_pool.tile([128, 4 * RW], F32, tag=f"g{d}")
                        for cc in range(4):
                            nc.tensor.transpose(tp[:, RW * cc:RW * cc + RW],
                                                h_[:, 128 * cc:128 * cc + 128],
                                                ident[0:RW, 0:RW])
                        nc.vector.tensor_copy(hT[d][:], tp[:])
                        if d == "f":
                            nc.sync.dma_start(tm[:, tx, 0:H], h_[:])
                        else:
                            nc.sync.dma_start(tm[:, tx, H:H2], h_[:])

            # ---------------- Phase 3: attention + enhancement ----------------
            with tc.tile_pool(name="a3", bufs=2) as a3, \
                 tc.tile_pool(name="a3s", bufs=2) as a3s, \
                 tc.tile_pool(name="eps", bufs=2, space="PSUM") as eps_pool, \
                 tc.tile_pool(name="tp3", bufs=2, space="PSUM") as tp3_pool, \
                 tc.tile_pool(name="ops", bufs=2, space="PSUM") as ops_pool:
                for n in range(PB):
                    a_tm = a3.tile([128, H2], F32, tag="a_tm")
                    nc.sync.dma_start(a_tm[:], tm[n, :, :])
                    b_tm = a3.tile([128, H2], F32, tag="b_tm")
                    nc.sync.dma_start(b_tm[:], tm[PB + n, :, :])
                    a_fm = a3.tile([128, H2], F32, tag="a_fm")
                    b_fm = a3.tile([128, H2], F32, tag="b_fm")
                    for src, dst in ((a_tm, a_fm), (b_tm, b_fm)):
                        for cc in range(8):
                            tp3 = tp3_pool.tile([128, 128], F32, tag="tp3")
                            nc.tensor.transpose(tp3[:], src[:, 128 * cc:128 * (cc + 1)],
                                                ident[:])
                            nc.scalar.copy(dst[:, 128 * cc:128 * (cc + 1)], tp3[:])
                    e_ps = eps_pool.tile([128, 128], F32, tag="e")
                    e2_ps = eps_pool.tile([128, 128], F32, tag="e")
                    for cc in range(8):
                        sl = slice(128 * cc, 128 * (cc + 1))
                        nc.tensor.matmul(e_ps[:], a_fm[:, sl], b_fm[:, sl],
                                         start=(cc == 0), stop=(cc == 7))
                    for cc in range(8):
                        sl = slice(128 * cc, 128 * (cc + 1))
                        nc.tensor.matmul(e2_ps[:], b_fm[:, sl], a_fm[:, sl],
                                         start=(cc == 0), stop=(cc == 7))
                    zs, rs = [], []
                    for eps in (e_ps, e2_ps):
                        m_ = a3s.tile([128, 1], F32, tag="m_")
                        nc.vector.tensor_reduce(m_[:], eps[:], axis=AX.X,
                                                op=ALU.max, negate=True)
                        z_ = a3s.tile([128, 128], F32, tag="z_")
                        s_ = a3s.tile([128, 1], F32, tag="s_")
                        nc.scalar.activation(z_[:], eps[:], AF.Exp, bias=m_[:],
                                             accum_out=s_[:])
                        r_ = a3s.tile([128, 1], F32, tag="r_")
                        nc.vector.reciprocal(r_[:], s_[:])
                        zt_ps = tp3_pool.tile([128, 128], F32, tag="tp3")
                        nc.tensor.transpose(zt_ps[:], z_[:], ident[:])
                        zt = a3s.tile([128, 128], F32, tag="zt")
                        nc.scalar.copy(zt[:], zt_ps[:])
                        zs.append(zt)
                        rs.append(r_)
                    tilded = []
                    for zt, r_, rhs_tm in ((zs[0], rs[0], b_tm), (zs[1], rs[1], a_tm)):
                        t_ps = ops_pool.tile([128, H2], F32, tag="t_ps")
                        for half in range(2):
                            sl = slice(512 * half, 512 * (half + 1))
                            nc.tensor.matmul(t_ps[:, sl], zt[:], rhs_tm[:, sl],
                                             start=True, stop=True)
                        til = a3.tile([128, H2], F32, tag="til")
                        nc.vector.tensor_scalar_mul(til[:], t_ps[:], r_[:])
                        tilded.append(til)
                    for bar, til, outd in ((a_tm, tilded[0], outA_d),
                                           (b_tm, tilded[1], outB_d)):
                        nc.sync.dma_start(outd.ap()[n, :, 0:H2], bar[:])
                        nc.sync.dma_start(outd.ap()[n, :, H2:2 * H2], til[:])
                        df = a3.tile([128, H2], F32, tag="df")
                        nc.gpsimd.tensor_sub(df[:], bar[:], til[:])
                        nc.sync.dma_start(outd.ap()[n, :, 2 * H2:3 * H2], df[:])
                        pr = a3.tile([128, H2], F32, tag="pr")
                        nc.vector.tensor_mul(pr[:], bar[:], til[:])
                        nc.sync.dma_start(outd.ap()[n, :, 3 * H2:4 * H2], pr[:])

    nc.compile()
    return nc


def _get_nc():
    if "nc" not in _CACHE:
        _CACHE["nc"] = _build()
    return _CACHE["nc"]


def prep_in_maps(inputs):
    A = np.asarray(inputs["A"])
    B = np.asarray(inputs["B"])
    embed = np.asarray(inputs["embed"], dtype=np.float32)
    # permute pytorch gate order [i,f,g,o] -> [i,f,o,g]
    perm = np.concatenate([np.arange(0, 2 * H), np.arange(3 * H, 4 * H),
                           np.arange(2 * H, 3 * H)])
    wmat, bmat = {}, {}
    for d in "fb":
        suf = "_f" if d == "f" else "_b"
        wihT = np.ascontiguousarray(
            np.asarray(inputs["Wih" + suf], dtype=np.float32)[perm].T)
        whhT = np.ascontiguousarray(
            np.asarray(inputs["Whh" + suf], dtype=np.float32)[perm].T)
        bias = (np.asarray(inputs["bih" + suf], dtype=np.float32)
                + np.asarray(inputs["bhh" + suf], dtype=np.float32))[perm]
        bias_bc = np.ascontiguousarray(
            np.broadcast_to(bias[None, :], (128, G4)), dtype=np.float32)
        wmat[d] = (wihT, whhT)
        bmat[d] = bias_bc

    xa = embed[A]    # [BSZ, T, E]
    xb = embed[B]

    in_maps = []
    for c in range(NCORES):
        sl = slice(PB * c, PB * (c + 1))
        xc = np.concatenate([xa[sl], xb[sl]], axis=0)          # [RW, T, E]
        xT = np.ascontiguousarray(
            xc.transpose(2, 0, 1).reshape(E, RW * T), dtype=np.float32)
        in_maps.append({
            "xT": xT,
            "wihT_f": wmat["f"][0], "whhT_f": wmat["f"][1], "bias_f": bmat["f"],
            "wihT_b": wmat["b"][0], "whhT_b": wmat["b"][1], "bias_b": bmat["b"],
        })
    return in_maps


def kernel(**inputs):
    from concourse.bass_utils import run_bass_kernel_spmd

    in_maps = prep_in_maps(inputs)
    nc = _get_nc()
    res = run_bass_kernel_spmd(nc, in_maps, core_ids=list(range(NCORES)))
    outA = np.concatenate([res.results[c]["outA"] for c in range(NCORES)], axis=0)
    outB = np.concatenate([res.results[c]["outB"] for c in range(NCORES)], axis=0)
    return outA, outB
